# revision 53
# baseline (speedup 1.0000x reference)
"""Multi-head self-attention (B=2, S=2048, D=1024, H=16, causal) on 8 trn2 cores.

Sharding: core c handles batch b = c//4 and 4 heads (c%4)*4 .. +4.
Per-core device program (all-transposed layout, no on-chip transposes):
  QT[dh,S] = Wq^T x^T, KT = Wk^T x^T   (heads stacked in pairs of 2 -> 128 parts)
  V[S,dvh] = (x^T)^T Wv, with a ones column appended (V\' is [128,65] per block)
  per head, per q-half, per key block kb:
    ST[kp, q] = K Q^T for q >= 128*kb   (causal, scores transposed)
    exp on ScalarE (no max subtraction -- scores are provably < ~3 here);
    the diagonal block is masked multiplicatively on the bf16 exp output
    outT[65, q] += V\'^T exp(ST)        (row 64 = softmax denominator)
Host: out = (outT[:64]/outT[64]).T + bv, reassembled into [B,S,H*dvh].
"""

import os
import sys

import numpy as np
import ml_dtypes

for _p in ("/opt/trn_rl_repo",):
    if _p not in sys.path and os.path.isdir(_p):
        sys.path.insert(0, _p)

B, S, D = 2, 2048, 1024
H = 16
DH = 64           # qk head dim
DVH = 64          # v head dim
HPC = 4           # heads per core
NCORES = 8
SCALE = 1.0 / 8.0  # 1/sqrt(dvh)
QH = 512           # q-quarter size

BF16 = ml_dtypes.bfloat16

_CACHE = {}


def _kernel_salt():
    # The PJRT neff cache keys on the HLO module hash, which does NOT
    # include the bass_exec kernel payload -- a stale cache silently runs
    # an OLD kernel. Salting an input tensor name with a digest of this
    # file forces a distinct HLO per kernel version.
    import hashlib
    try:
        with open(__file__, "rb") as f:
            return hashlib.sha1(f.read()).hexdigest()[:10]
    except OSError:
        return "nosalt"


def _build_program(repeat=1):
    import concourse.tile as tile
    from concourse import bacc, mybir

    dt = mybir.dt
    nc = bacc.Bacc("TRN2", target_bir_lowering=False, debug=False,
                   num_devices=NCORES)

    salt_d = nc.dram_tensor(f"salt_{_kernel_salt()}", [1, 2], dt.float32,
                            kind="ExternalInput").ap()

    xt_d = nc.dram_tensor("xt", [8, 128, S], dt.bfloat16, kind="ExternalInput").ap()
    wq_d = nc.dram_tensor("wq", [8, 128, 256], dt.bfloat16, kind="ExternalInput").ap()
    wk_d = nc.dram_tensor("wk", [8, 128, 256], dt.bfloat16, kind="ExternalInput").ap()
    wv_d = nc.dram_tensor("wv", [8, 128, 256], dt.bfloat16, kind="ExternalInput").ap()
    bq_d = nc.dram_tensor("bq", [128, 2], dt.float32, kind="ExternalInput").ap()
    bk_d = nc.dram_tensor("bk", [128, 2], dt.float32, kind="ExternalInput").ap()
    am_d = nc.dram_tensor("amask", [128, 128], dt.bfloat16, kind="ExternalInput").ap()
    out_d = nc.dram_tensor("out", [HPC, 65, S], dt.float32, kind="ExternalOutput").ap()

    for _ in range(repeat):
        _build_body(nc, tile, mybir,
                    xt_d, wq_d, wk_d, wv_d, bq_d, bk_d, am_d, out_d, salt_d)

    nc.compile()
    return nc


def _build_body(nc, tile, mybir, xt_d, wq_d, wk_d, wv_d, bq_d, bk_d, am_d,
                out_d, salt_d):
    dt = mybir.dt
    Exp = mybir.ActivationFunctionType.Exp

    with tile.TileContext(nc) as tc:
        with (
            tc.tile_pool(name="const", bufs=1) as const,
            tc.tile_pool(name="expp", bufs=20) as expp,
            tc.tile_pool(name="osb", bufs=2) as osb,
        ):
            xt_sb = const.tile([128, 8, S], dt.bfloat16)
            wq_sb = const.tile([128, 8, 256], dt.bfloat16)
            wk_sb = const.tile([128, 8, 256], dt.bfloat16)
            wv_sb = const.tile([128, 8, 256], dt.bfloat16)
            bq_sb = const.tile([128, 2], dt.float32)
            bk_sb = const.tile([128, 2], dt.float32)
            am_sb = const.tile([128, 128], dt.bfloat16)
            qt_sb = const.tile([128, 2, S], dt.bfloat16)
            kt_sb = const.tile([128, 2, S], dt.bfloat16)
            v_sb = const.tile([128, 16, HPC, 65], dt.bfloat16)
            salt_sb = const.tile([1, 2], dt.float32)

            # DMA issue order matters: HWDGE descriptor-gen serializes per
            # ring (~0.6us each), so issue first-needed tensors first and
            # alternate between the two HWDGE rings (sync + vector). The
            # scalar(Act) ring is NOT used: Act is a near-critical engine
            # and each dma_start costs it ~0.5us of issue time.
            # wq split kc0-first: the very first matmul needs only wq[kc0]
            # + xt[kc0, ct0]; small transfers land much earlier. The first
            # projection accumulates over kc 0-7 in ~2us, so xt ct0 chunks
            # are spread over both HWDGE rings + gpsimd SWDGE in kc order.
            nc.sync.dma_start(wq_sb[:, 0, :], wq_d[0].rearrange("p n -> p n"))
            nc.scalar.dma_start(
                wq_sb[:, 2:8, :], wq_d[2:8].rearrange("c p n -> p c n"))
            nc.sync.dma_start(xt_sb[:, 0, 0:512], xt_d[0, :, 0:512])
            nc.scalar.dma_start(xt_sb[:, 1, 0:512], xt_d[1, :, 0:512])
            nc.sync.dma_start(wq_sb[:, 1, :], wq_d[1].rearrange("p n -> p n"))
            nc.sync.dma_start(xt_sb[:, 2, 0:512], xt_d[2, :, 0:512])
            nc.scalar.dma_start(wk_sb[:, 0, :], wk_d[0].rearrange("p n -> p n"))
            nc.sync.dma_start(bq_sb[:, :], bq_d)
            nc.scalar.dma_start(xt_sb[:, 3, 0:512], xt_d[3, :, 0:512])
            nc.sync.dma_start(xt_sb[:, 4, 0:512], xt_d[4, :, 0:512])
            nc.scalar.dma_start(
                wk_sb[:, 1:8, :], wk_d[1:8].rearrange("c p n -> p c n"))
            nc.sync.dma_start(xt_sb[:, 6, 0:512], xt_d[6, :, 0:512])
            nc.gpsimd.dma_start(xt_sb[:, 5, 0:512], xt_d[5, :, 0:512])
            nc.gpsimd.dma_start(xt_sb[:, 7, 0:512], xt_d[7, :, 0:512])
            nc.gpsimd.dma_start(bk_sb[:, :], bk_d)
            nc.gpsimd.dma_start(am_sb[:, :], am_d)
            nc.scalar.dma_start(wv_sb[:, :, :], wv_d.rearrange("c p n -> p c n"))
            # bulk xt for quarters 1-3, batched 4-chunk transfers: even kc
            # on the sync HWDGE ring, odd kc on gpsimd SWDGE so the Act
            # ring is free for exp and ring issue slots stay cheap. ct1 is
            # needed ~9us in (row 1 projections), so it leads.
            nc.sync.dma_start(
                xt_sb[:, 0:8:2, 512:1024],
                xt_d[0:8:2, :, 512:1024].rearrange("c p n -> p c n"))
            nc.gpsimd.dma_start(
                xt_sb[:, 1:8:2, 512:1024],
                xt_d[1:8:2, :, 512:1024].rearrange("c p n -> p c n"))
            for ct in (2, 3):
                nc.sync.dma_start(
                    xt_sb[:, 0:8:2, 512 * ct:512 * (ct + 1)],
                    xt_d[0:8:2, :, 512 * ct:512 * (ct + 1)].rearrange(
                        "c p n -> p c n"))
                nc.gpsimd.dma_start(
                    xt_sb[:, 1:8:2, 512 * ct:512 * (ct + 1)],
                    xt_d[1:8:2, :, 512 * ct:512 * (ct + 1)].rearrange(
                        "c p n -> p c n"))
            # keep the cache-salt tensor alive in the NEFF (see _kernel_salt);
            # issued last, it has no consumers
            nc.gpsimd.dma_start(salt_sb[:, :], salt_d)

            def proj_qk_ct(pool, tag, p, ct, which):
                dst_sb, w_sb, b_sb = ((qt_sb, wq_sb, bq_sb),
                                      (kt_sb, wk_sb, bk_sb))[which]
                ps = pool.tile([128, 512], dt.float32, tag=tag, name="ps")
                for kc in range(8):
                    nc.tensor.matmul(
                        ps,
                        w_sb[:, kc, 128 * p:128 * (p + 1)],
                        xt_sb[:, kc, 512 * ct:512 * (ct + 1)],
                        start=(kc == 0), stop=(kc == 7),
                    )
                nc.vector.tensor_scalar_add(
                    dst_sb[:, p, 512 * ct:512 * (ct + 1)], ps, b_sb[:, p:p + 1])

            def proj_v_sc(pool, tag, sc):
                # V: [S, 4 heads x 64] natural layout + ones col appended
                ps2 = pool.tile([128, HPC, 64], dt.float32, tag=tag, name="ps2")
                for kc in range(8):
                    nc.tensor.matmul(
                        ps2,
                        xt_sb[:, kc, 128 * sc:128 * (sc + 1)],
                        wv_sb[:, kc, :],
                        start=(kc == 0), stop=(kc == 7),
                    )
                nc.vector.tensor_copy(v_sb[:, sc, :, 0:64], ps2)

            nc.vector.memset(v_sb[:, :, :, 64], 1.0)
            # stp(2x2banks) + op(3) + pp(1) = 8 PSUM banks. st tiles span
            # two banks so one exp instruction covers two 512-wide key
            # blocks (halves Act per-instruction access overhead).
            stp = tc.alloc_tile_pool(name="stp", bufs=2, space="PSUM")
            op = tc.alloc_tile_pool(name="op", bufs=3, space="PSUM")
            pp = tc.alloc_tile_pool(name="pp", bufs=1, space="PSUM")

            # global software pipeline across all (head, q-quarter) units:
            # one rolling pending-AV queue so the exp->AV edge never drains
            from collections import deque
            pend = deque()
            cur_out = {}
            LAG = 26

            def emit_av_one():
                (u, h, h0, h1, kb, cq0, clen, isdiag, ext, is_last) = pend.popleft()
                if u not in cur_out:
                    cur_out[u] = op.tile([65, QH], dt.float32, tag="op",
                                         name="outp")
                outp = cur_out[u]
                segs = []
                s0 = cq0
                if isdiag:
                    segs.append((cq0, 128, True))
                    s0 = cq0 + 128
                while s0 < h1:
                    s1 = min((s0 // 512 + 1) * 512, h1)
                    segs.append((s0, s1 - s0, False))
                    s0 = s1
                final = h == 3 and h0 == 3 * QH
                for (g0, gl, isd) in segs:
                    nc.tensor.matmul(
                        outp[:, g0 - h0:g0 - h0 + gl],
                        v_sb[:, kb, h, :],
                        ext[:, g0 - cq0:g0 - cq0 + gl],
                        start=(kb == 0 and g0 % 512 == 0),
                        stop=(isd and kb % 4 == 3),
                        # final unit: columns are streamed out as each
                        # 128-block finishes accumulating (see below), so
                        # the group-completeness check must be bypassed.
                        # stop/group flags are sim-only; hardware PSUM
                        # accumulation is controlled by `start` alone.
                        skip_group_check=final,
                    )
                if final and kb >= 12:
                    # col block b=[kb-12] is final after kb's AV: stream it
                    # out now so only a 128-col chain trails the last AV
                    blk = kb - 12
                    c0, c1 = 128 * blk, 128 * blk + 128
                    if "fin_ot" not in cur_out:
                        cur_out["fin_ot"] = osb.tile([65, QH], dt.float32,
                                                     tag="ot", name="ot")
                    ot = cur_out["fin_ot"]
                    nc.vector.tensor_copy(ot[:, c0:c1], outp[:, c0:c1])
                    ring = nc.sync if blk % 2 == 0 else nc.scalar
                    ring.dma_start(out_d[h, :, h0 + c0:h0 + c1], ot[:, c0:c1])
                    if kb == 15:
                        del cur_out["fin_ot"]
                        del cur_out[u]
                elif is_last:
                    ot = osb.tile([65, QH], dt.float32, tag="ot", name="ot")
                    nc.vector.tensor_copy(ot, outp)
                    nc.sync.dma_start(out_d[h, :, h0:h0 + QH], ot)
                    del cur_out[u]

            def attn_chunk(h, qh, group, last_kb):
                p, hi = h // 2, h % 2
                base = 64 * hi
                h0, h1 = QH * qh, QH * (qh + 1)
                u = (h, qh)
                cq0s = [max(128 * kb, h0) for kb in group]
                clens = [h1 - c for c in cq0s]
                width = sum(clens)
                st = stp.tile([128, width], dt.float32, tag="st", name="st")
                off = 0
                for kb, cq0, clen in zip(group, cq0s, clens):
                    n0 = 0
                    while n0 < clen:
                        nl = min(512, clen - n0)
                        nc.tensor.matmul(
                            st[:, off + n0:off + n0 + nl],
                            kt_sb[base:base + 64, p, 128 * kb:128 * kb + 128],
                            qt_sb[base:base + 64, p, cq0 + n0:cq0 + n0 + nl],
                            start=True, stop=True,
                        )
                        n0 += nl
                    off += clen
                ext = expp.tile([128, width], dt.bfloat16, tag="ex", name="ext")
                nc.scalar.activation(ext, st, Exp, scale=SCALE)
                off = 0
                for kb, cq0, clen in zip(group, cq0s, clens):
                    isdiag = 128 * kb >= h0
                    if isdiag:
                        nc.vector.tensor_mul(ext[:, off:off + 128],
                                             ext[:, off:off + 128], am_sb)
                    pend.append((u, h, h0, h1, kb, cq0, clen, isdiag,
                                 ext[:, off:off + clen], kb == last_kb))
                    off += clen
                while len(pend) > LAG:
                    emit_av_one()

            def attn_drain():
                while pend:
                    emit_av_one()

            # Software-pipelined schedule. Row qh = all 4 heads' attention
            # on query quarter qh; projections for row qh+1 are woven in as
            # PE filler so the Act engine (exp) stays fed and the PE never
            # waits on the stp pool when Act lags (exp is ~12% slower than
            # scores+AV during pure-attention stretches).
            def unit_groups(qh):
                nkb = 4 * qh + 4
                groups = [[kb, kb + 1] for kb in range(0, 4 * qh, 2)]
                groups += [[kb] for kb in range(4 * qh, nkb)]
                return groups

            def section(h_pair, qh):
                # two heads' chunks interleaved; keeps 2 outp units live
                nkb = 4 * qh + 4
                gs = unit_groups(qh)
                out = []
                for g in gs:
                    for h in h_pair:
                        out.append((h, qh, g, nkb - 1))
                return out

            def weave(chunks, fillers):
                """fillers: list of (pos, closure); closure is emitted right
                after chunk index == pos (exact)."""
                out = {i: [] for i in range(len(chunks))}
                for pos, fn in fillers:
                    out[min(pos, len(chunks) - 1)].append(fn)
                for i, (h, qh, g, last_kb) in enumerate(chunks):
                    attn_chunk(h, qh, g, last_kb)
                    for fn in out[i]:
                        fn()

            def u_qk(p, ct, which):
                return lambda: proj_qk_ct(pp, "pp", p, ct, which)

            def u_v(sc):
                return lambda: proj_v_sc(pp, "pp", sc)

            # Every row is two sections: [h0 x h1 interleaved] then
            # [h2 x h3]. p1 projections are legal filler for the h2/h3
            # sections, so every section gets enough PE filler to cover
            # Act's ~1.22x per-chunk cost and the PE never stalls dry.
            proj_qk_ct(pp, "pp", 0, 0, 0)
            proj_qk_ct(pp, "pp", 0, 0, 1)
            # r0A (8 chunks): h0/h1 q0. v_i before kb_i AV flush.
            weave(section((0, 1), 0), [
                (2, u_v(0)), (3, u_v(1)), (5, u_v(2)), (6, u_v(3)),
                (6, u_qk(1, 0, 0)), (7, u_qk(1, 0, 1)),
            ])
            # r0B (8): h2/h3 q0; fill with row1 p0 + v4/v5
            weave(section((2, 3), 0), [
                (1, u_qk(0, 1, 0)), (3, u_qk(0, 1, 1)),
                (5, u_v(4)), (7, u_v(5)),
            ])
            # r1A (12): h0/h1 q1; v6/v7 due by kb6/7 AVs; p0 q2/k2 + p1 q1/k1
            # (p1 needed by r1B chunk 0)
            weave(section((0, 1), 1), [
                (2, u_v(6)), (4, u_v(7)),
                (6, u_qk(0, 2, 0)), (8, u_qk(1, 1, 0)), (10, u_qk(1, 1, 1)),
            ])
            # r1B (12): h2/h3 q1
            weave(section((2, 3), 1), [
                (2, u_v(8)), (4, u_qk(0, 2, 1)),
                (6, u_v(9)), (8, u_qk(1, 2, 0)), (10, u_qk(1, 2, 1)),
            ])
            # r2A (16): h0/h1 q2
            weave(section((0, 1), 2), [
                (0, u_qk(0, 3, 0)), (2, u_v(10)), (4, u_v(11)),
                (8, u_qk(0, 3, 1)),
            ])
            # r2B (16): h2/h3 q2
            weave(section((2, 3), 2), [
                (4, u_qk(1, 3, 0)),
            ])
            # r3A (20): h0/h1 q3; v12-15 due before h0 kb12-15 AVs (~c16+)
            weave(section((0, 1), 3), [
                (0, u_v(12)), (2, u_v(13)), (4, u_v(14)), (6, u_v(15)),
            ])
            # r3B (20): h2/h3 q3; p1 k3 due before h2 kb12 (chunk 12)
            weave(section((2, 3), 3), [
                (6, u_qk(1, 3, 1)),
            ])
            attn_drain()
            pp.release()
            op.release()
            stp.release()


def _get_program():
    if "nc" not in _CACHE:
        _CACHE["nc"] = _build_program()
    return _CACHE["nc"]


def make_in_maps(x, Wqk, bqk, Wv, bv):
    ii, jj = np.meshgrid(np.arange(128), np.arange(128), indexing="ij")
    amask = np.where(ii <= jj, 1.0, 0.0).astype(BF16)
    in_maps = []
    for c in range(NCORES):
        b, g = divmod(c, 4)
        cols = slice(256 * g, 256 * (g + 1))
        xt = np.ascontiguousarray(x[b].T).astype(BF16).reshape(8, 128, S)
        wq = np.ascontiguousarray(Wqk[:, cols]).astype(BF16).reshape(8, 128, 256)
        wk = np.ascontiguousarray(Wqk[:, D:][:, cols]).astype(BF16).reshape(8, 128, 256)
        wv = np.ascontiguousarray(Wv[:, cols]).astype(BF16).reshape(8, 128, 256)
        bq = np.ascontiguousarray(bqk[cols].reshape(2, 128).T).astype(np.float32)
        bk = np.ascontiguousarray(bqk[D:][cols].reshape(2, 128).T).astype(np.float32)
        in_maps.append({"xt": xt, "wq": wq, "wk": wk, "wv": wv,
                        "bq": bq, "bk": bk, "amask": amask,
                        f"salt_{_kernel_salt()}": np.zeros((1, 2), np.float32)})
    return in_maps


def assemble(per_core_out, bv):
    out = np.empty((B, S, H * DVH), np.float32)
    for c in range(NCORES):
        b, g = divmod(c, 4)
        o = per_core_out[c]  # [HPC, 65, S]
        for hh in range(HPC):
            hg = HPC * g + hh
            a = o[hh, :64, :] / o[hh, 64:65, :]
            out[b, :, DVH * hg:DVH * (hg + 1)] = a.T + bv[DVH * hg:DVH * (hg + 1)]
    return out


def kernel(x, Wqk, bqk, Wv, bv):
    from concourse.bass_utils import run_bass_kernel_spmd

    nc = _get_program()
    in_maps = make_in_maps(np.asarray(x, np.float32), np.asarray(Wqk, np.float32),
                           np.asarray(bqk, np.float32), np.asarray(Wv, np.float32),
                           np.asarray(bv, np.float32))
    trace = os.environ.get("MHA_TRACE", "0") == "1"
    try:
        res = run_bass_kernel_spmd(nc, in_maps, list(range(NCORES)), trace=trace)
    except Exception:
        if not trace:
            raise
        # trace path unavailable (e.g. no NTFF hook on this axon client)
        res = run_bass_kernel_spmd(nc, in_maps, list(range(NCORES)), trace=False)
    _CACHE["last_result"] = res
    return assemble([r["out"] for r in res.results], np.asarray(bv, np.float32))



# revision 67
# speedup vs baseline: 3.0674x; 3.0674x over previous
"""Multi-head self-attention (B=2, S=2048, D=1024, H=16, causal) on 8 trn2 cores.

Sharding: core c handles batch b = c//4 and 4 heads (c%4)*4 .. +4.
Per-core device program (all-transposed layout, no on-chip transposes):
  QT[dh,S] = Wq^T x^T, KT = Wk^T x^T   (heads stacked in pairs of 2 -> 128 parts)
  V[S,dvh] = (x^T)^T Wv, with a ones column appended (V\' is [128,65] per block)
  per head, per q-quarter, per key block kb:
    ST[kp, q] = K Q^T for q >= 128*kb   (causal, scores transposed)
    exp on ScalarE (no max subtraction -- scores are provably < ~3 here);
    adjacent full-512 key blocks share one [128,1024] two-bank PSUM tile so
    a single exp instruction covers both (halves Act access overhead);
    the diagonal block is masked multiplicatively on the bf16 exp output
    outT[65, q] += V\'^T exp(ST)        (row 64 = softmax denominator)
Host: out = (outT[:64]/outT[64]).T + bv, reassembled into [B,S,H*dvh].

Schedule (the perf-critical part): rows = q-quarters, each row split into
two head-pair sections [h0 x h1] then [h2 x h3] (caps live PSUM output
tiles at 2-3). Projections are software-pipelined one row ahead and woven
between attention chunks as PE filler, sized per section so the PE never
runs dry while the Act engine (exp, ~1.2x slower than scores+AV in pure
attention) catches up. A deep pending-AV queue (LAG=26) decouples the
exp->AV edge; PSUM: stp 2x2banks (paired scores) + op 2 (outputs) + pp 2
(projection double-buffer) = 8 banks. The final unit streams its output
per 128-col block so only a ~2.5us drain trails the last AV. CoreSim:
109.4us span, PE 99.1us busy (90.6%), vs 133.6us/75.5% for the v1 layout.
"""

import os
import sys

import numpy as np
import ml_dtypes

for _p in ("/opt/trn_rl_repo",):
    if _p not in sys.path and os.path.isdir(_p):
        sys.path.insert(0, _p)

B, S, D = 2, 2048, 1024
H = 16
DH = 64           # qk head dim
DVH = 64          # v head dim
HPC = 4           # heads per core
NCORES = 8
SCALE = 1.0 / 8.0  # 1/sqrt(dvh)
QH = 512           # q-quarter size

BF16 = ml_dtypes.bfloat16

_CACHE = {}


def _kernel_salt():
    # The PJRT neff cache keys on the HLO module hash, which does NOT
    # include the bass_exec kernel payload -- a stale cache silently runs
    # an OLD kernel. Salting an input tensor name with a digest of this
    # file forces a distinct HLO per kernel version.
    import hashlib
    try:
        with open(__file__, "rb") as f:
            return hashlib.sha1(f.read()).hexdigest()[:10]
    except OSError:
        return "nosalt"


def _build_program(repeat=1):
    import concourse.tile as tile
    from concourse import bacc, mybir

    dt = mybir.dt
    nc = bacc.Bacc("TRN2", target_bir_lowering=False, debug=False,
                   num_devices=NCORES)

    salt_d = nc.dram_tensor(f"salt_{_kernel_salt()}", [1, 2], dt.float32,
                            kind="ExternalInput").ap()

    xt_d = nc.dram_tensor("xt", [8, 128, S], dt.bfloat16, kind="ExternalInput").ap()
    wq_d = nc.dram_tensor("wq", [8, 128, 256], dt.bfloat16, kind="ExternalInput").ap()
    wk_d = nc.dram_tensor("wk", [8, 128, 256], dt.bfloat16, kind="ExternalInput").ap()
    wv_d = nc.dram_tensor("wv", [8, 128, 256], dt.bfloat16, kind="ExternalInput").ap()
    bq_d = nc.dram_tensor("bq", [128, 2], dt.float32, kind="ExternalInput").ap()
    bk_d = nc.dram_tensor("bk", [128, 2], dt.float32, kind="ExternalInput").ap()
    am_d = nc.dram_tensor("amask", [128, 128], dt.bfloat16, kind="ExternalInput").ap()
    out_d = nc.dram_tensor("out", [HPC, 65, S], dt.float32, kind="ExternalOutput").ap()

    for _ in range(repeat):
        _build_body(nc, tile, mybir,
                    xt_d, wq_d, wk_d, wv_d, bq_d, bk_d, am_d, out_d, salt_d)

    nc.compile()
    return nc


def _build_body(nc, tile, mybir, xt_d, wq_d, wk_d, wv_d, bq_d, bk_d, am_d,
                out_d, salt_d):
    dt = mybir.dt
    Exp = mybir.ActivationFunctionType.Exp

    with tile.TileContext(nc) as tc:
        with (
            tc.tile_pool(name="const", bufs=1) as const,
            tc.tile_pool(name="expp", bufs=20) as expp,
            tc.tile_pool(name="osb", bufs=2) as osb,
        ):
            xt_sb = const.tile([128, 8, S], dt.bfloat16)
            wq_sb = const.tile([128, 8, 256], dt.bfloat16)
            wk_sb = const.tile([128, 8, 256], dt.bfloat16)
            wv_sb = const.tile([128, 8, 256], dt.bfloat16)
            bq_sb = const.tile([128, 2], dt.float32)
            bk_sb = const.tile([128, 2], dt.float32)
            am_sb = const.tile([128, 128], dt.bfloat16)
            qt_sb = const.tile([128, 2, S], dt.bfloat16)
            kt_sb = const.tile([128, 2, S], dt.bfloat16)
            v_sb = const.tile([128, 16, HPC, 65], dt.bfloat16)
            salt_sb = const.tile([1, 2], dt.float32)

            # DMA issue order matters: HWDGE descriptor-gen serializes per
            # ring (~0.6us each), so issue first-needed tensors first and
            # alternate between the two HWDGE rings (sync + vector). The
            # scalar(Act) ring is NOT used: Act is a near-critical engine
            # and each dma_start costs it ~0.5us of issue time.
            # wq split kc0-first: the very first matmul needs only wq[kc0]
            # + xt[kc0, ct0]; small transfers land much earlier. The first
            # projection accumulates over kc 0-7 in ~2us, so xt ct0 chunks
            # are spread over both HWDGE rings + gpsimd SWDGE in kc order.
            nc.sync.dma_start(wq_sb[:, 0, :], wq_d[0].rearrange("p n -> p n"))
            nc.scalar.dma_start(
                wq_sb[:, 1:8, :], wq_d[1:8].rearrange("c p n -> p c n"))
            nc.sync.dma_start(xt_sb[:, 0, 0:512], xt_d[0, :, 0:512])
            nc.scalar.dma_start(xt_sb[:, 1, 0:512], xt_d[1, :, 0:512])
            nc.sync.dma_start(xt_sb[:, 2, 0:512], xt_d[2, :, 0:512])
            nc.scalar.dma_start(wk_sb[:, 0, :], wk_d[0].rearrange("p n -> p n"))
            nc.sync.dma_start(bq_sb[:, :], bq_d)
            nc.scalar.dma_start(xt_sb[:, 3, 0:512], xt_d[3, :, 0:512])
            nc.sync.dma_start(xt_sb[:, 4, 0:512], xt_d[4, :, 0:512])
            nc.scalar.dma_start(
                wk_sb[:, 1:8, :], wk_d[1:8].rearrange("c p n -> p c n"))
            nc.sync.dma_start(xt_sb[:, 6, 0:512], xt_d[6, :, 0:512])
            nc.gpsimd.dma_start(xt_sb[:, 5, 0:512], xt_d[5, :, 0:512])
            nc.gpsimd.dma_start(xt_sb[:, 7, 0:512], xt_d[7, :, 0:512])
            nc.gpsimd.dma_start(bk_sb[:, :], bk_d)
            nc.gpsimd.dma_start(am_sb[:, :], am_d)
            nc.scalar.dma_start(wv_sb[:, :, :], wv_d.rearrange("c p n -> p c n"))
            # bulk xt for quarters 1-3, batched 4-chunk transfers: even kc
            # on the sync HWDGE ring, odd kc on gpsimd SWDGE so the Act
            # ring is free for exp and ring issue slots stay cheap. ct1 is
            # needed ~9us in (row 1 projections), so it leads.
            nc.sync.dma_start(
                xt_sb[:, 0:8:2, 512:1024],
                xt_d[0:8:2, :, 512:1024].rearrange("c p n -> p c n"))
            nc.gpsimd.dma_start(
                xt_sb[:, 1:8:2, 512:1024],
                xt_d[1:8:2, :, 512:1024].rearrange("c p n -> p c n"))
            for ct in (2, 3):
                nc.sync.dma_start(
                    xt_sb[:, 0:8:2, 512 * ct:512 * (ct + 1)],
                    xt_d[0:8:2, :, 512 * ct:512 * (ct + 1)].rearrange(
                        "c p n -> p c n"))
                nc.gpsimd.dma_start(
                    xt_sb[:, 1:8:2, 512 * ct:512 * (ct + 1)],
                    xt_d[1:8:2, :, 512 * ct:512 * (ct + 1)].rearrange(
                        "c p n -> p c n"))
            # keep the cache-salt tensor alive in the NEFF (see _kernel_salt);
            # issued last, it has no consumers
            nc.gpsimd.dma_start(salt_sb[:, :], salt_d)

            def proj_qk_ct(pool, tag, p, ct, which):
                dst_sb, w_sb, b_sb = ((qt_sb, wq_sb, bq_sb),
                                      (kt_sb, wk_sb, bk_sb))[which]
                ps = pool.tile([128, 512], dt.float32, tag=tag, name="ps")
                for kc in range(8):
                    nc.tensor.matmul(
                        ps,
                        w_sb[:, kc, 128 * p:128 * (p + 1)],
                        xt_sb[:, kc, 512 * ct:512 * (ct + 1)],
                        start=(kc == 0), stop=(kc == 7),
                    )
                nc.vector.tensor_scalar_add(
                    dst_sb[:, p, 512 * ct:512 * (ct + 1)], ps, b_sb[:, p:p + 1])

            def proj_v_sc(pool, tag, sc):
                # V: [S, 4 heads x 64] natural layout + ones col appended
                ps2 = pool.tile([128, HPC, 64], dt.float32, tag=tag, name="ps2")
                for kc in range(8):
                    nc.tensor.matmul(
                        ps2,
                        xt_sb[:, kc, 128 * sc:128 * (sc + 1)],
                        wv_sb[:, kc, :],
                        start=(kc == 0), stop=(kc == 7),
                    )
                nc.vector.tensor_copy(v_sb[:, sc, :, 0:64], ps2)

            nc.vector.memset(v_sb[:, :, :, 64], 1.0)
            # stp(2x2banks) + op(2) + pp(2) = 8 PSUM banks. st tiles span
            # two banks so one exp instruction covers two 512-wide key
            # blocks (halves Act per-instruction access overhead); pp is
            # double-buffered so projection units never serialize against
            # their own bias-add/copy drain.
            stp = tc.alloc_tile_pool(name="stp", bufs=2, space="PSUM")
            op = tc.alloc_tile_pool(name="op", bufs=2, space="PSUM")
            pp = tc.alloc_tile_pool(name="pp", bufs=2, space="PSUM")

            # global software pipeline across all (head, q-quarter) units:
            # one rolling pending-AV queue so the exp->AV edge never drains
            from collections import deque
            pend = deque()
            cur_out = {}
            LAG = 26

            def emit_av_one():
                (u, h, h0, h1, kb, cq0, clen, isdiag, ext, is_last) = pend.popleft()
                if u not in cur_out:
                    cur_out[u] = op.tile([65, QH], dt.float32, tag="op",
                                         name="outp")
                outp = cur_out[u]
                segs = []
                s0 = cq0
                if isdiag:
                    segs.append((cq0, 128, True))
                    s0 = cq0 + 128
                while s0 < h1:
                    s1 = min((s0 // 512 + 1) * 512, h1)
                    segs.append((s0, s1 - s0, False))
                    s0 = s1
                final = h == 3 and h0 == 3 * QH
                for (g0, gl, isd) in segs:
                    nc.tensor.matmul(
                        outp[:, g0 - h0:g0 - h0 + gl],
                        v_sb[:, kb, h, :],
                        ext[:, g0 - cq0:g0 - cq0 + gl],
                        start=(kb == 0 and g0 % 512 == 0),
                        stop=(isd and kb % 4 == 3),
                        # final unit: columns are streamed out as each
                        # 128-block finishes accumulating (see below), so
                        # the group-completeness check must be bypassed.
                        # stop/group flags are sim-only; hardware PSUM
                        # accumulation is controlled by `start` alone.
                        skip_group_check=final,
                    )
                if final and kb >= 12:
                    # col block b=[kb-12] is final after kb's AV: stream it
                    # out now so only a 128-col chain trails the last AV
                    blk = kb - 12
                    c0, c1 = 128 * blk, 128 * blk + 128
                    if "fin_ot" not in cur_out:
                        cur_out["fin_ot"] = osb.tile([65, QH], dt.float32,
                                                     tag="ot", name="ot")
                    ot = cur_out["fin_ot"]
                    nc.vector.tensor_copy(ot[:, c0:c1], outp[:, c0:c1])
                    ring = nc.sync if blk % 2 == 0 else nc.scalar
                    ring.dma_start(out_d[h, :, h0 + c0:h0 + c1], ot[:, c0:c1])
                    if kb == 15:
                        del cur_out["fin_ot"]
                        del cur_out[u]
                elif is_last:
                    ot = osb.tile([65, QH], dt.float32, tag="ot", name="ot")
                    nc.vector.tensor_copy(ot, outp)
                    nc.sync.dma_start(out_d[h, :, h0:h0 + QH], ot)
                    del cur_out[u]

            def attn_chunk(h, qh, group, last_kb):
                p, hi = h // 2, h % 2
                base = 64 * hi
                h0, h1 = QH * qh, QH * (qh + 1)
                u = (h, qh)
                cq0s = [max(128 * kb, h0) for kb in group]
                clens = [h1 - c for c in cq0s]
                width = sum(clens)
                st = stp.tile([128, width], dt.float32, tag="st", name="st")
                off = 0
                for kb, cq0, clen in zip(group, cq0s, clens):
                    n0 = 0
                    while n0 < clen:
                        nl = min(512, clen - n0)
                        nc.tensor.matmul(
                            st[:, off + n0:off + n0 + nl],
                            kt_sb[base:base + 64, p, 128 * kb:128 * kb + 128],
                            qt_sb[base:base + 64, p, cq0 + n0:cq0 + n0 + nl],
                            start=True, stop=True,
                        )
                        n0 += nl
                    off += clen
                ext = expp.tile([128, width], dt.bfloat16, tag="ex", name="ext")
                nc.scalar.activation(ext, st, Exp, scale=SCALE)
                off = 0
                for kb, cq0, clen in zip(group, cq0s, clens):
                    isdiag = 128 * kb >= h0
                    if isdiag:
                        nc.vector.tensor_mul(ext[:, off:off + 128],
                                             ext[:, off:off + 128], am_sb)
                    pend.append((u, h, h0, h1, kb, cq0, clen, isdiag,
                                 ext[:, off:off + clen], kb == last_kb))
                    off += clen
                while len(pend) > LAG:
                    emit_av_one()

            def attn_drain():
                while pend:
                    emit_av_one()

            # Software-pipelined schedule. Row qh = all 4 heads' attention
            # on query quarter qh; projections for row qh+1 are woven in as
            # PE filler so the Act engine (exp) stays fed and the PE never
            # waits on the stp pool when Act lags (exp is ~12% slower than
            # scores+AV during pure-attention stretches).
            def unit_groups(qh):
                nkb = 4 * qh + 4
                groups = [[kb, kb + 1] for kb in range(0, 4 * qh, 2)]
                groups += [[kb] for kb in range(4 * qh, nkb)]
                return groups

            def section(h_pair, qh):
                # two heads' chunks interleaved; keeps 2 outp units live
                nkb = 4 * qh + 4
                gs = unit_groups(qh)
                out = []
                for g in gs:
                    for h in h_pair:
                        out.append((h, qh, g, nkb - 1))
                return out

            def weave(chunks, fillers):
                """fillers: list of (pos, closure); closure is emitted right
                after chunk index == pos (exact)."""
                out = {i: [] for i in range(len(chunks))}
                for pos, fn in fillers:
                    out[min(pos, len(chunks) - 1)].append(fn)
                for i, (h, qh, g, last_kb) in enumerate(chunks):
                    attn_chunk(h, qh, g, last_kb)
                    for fn in out[i]:
                        fn()

            def u_qk(p, ct, which):
                return lambda: proj_qk_ct(pp, "pp", p, ct, which)

            def u_v(sc):
                return lambda: proj_v_sc(pp, "pp", sc)

            # Every row is two sections: [h0 x h1 interleaved] then
            # [h2 x h3]. p1 projections are legal filler for the h2/h3
            # sections, so every section gets enough PE filler to cover
            # Act's ~1.22x per-chunk cost and the PE never stalls dry.
            proj_qk_ct(pp, "pp", 0, 0, 0)
            proj_qk_ct(pp, "pp", 0, 0, 1)
            # r0A (8 chunks): h0/h1 q0. v_i before kb_i AV flush.
            weave(section((0, 1), 0), [
                (2, u_v(0)), (3, u_v(1)), (5, u_v(2)), (6, u_v(3)),
                (6, u_qk(1, 0, 0)), (7, u_qk(1, 0, 1)),
            ])
            # r0B (8): h2/h3 q0; fill with row1 p0 + v4/v5
            weave(section((2, 3), 0), [
                (1, u_qk(0, 1, 0)), (3, u_qk(0, 1, 1)),
                (5, u_v(4)), (7, u_v(5)),
            ])
            # r1A (12): h0/h1 q1; v6/v7 due by kb6/7 AVs; p0 q2/k2 + p1 q1/k1
            # (p1 needed by r1B chunk 0)
            weave(section((0, 1), 1), [
                (2, u_v(6)), (4, u_v(7)),
                (6, u_qk(0, 2, 0)), (8, u_qk(1, 1, 0)), (10, u_qk(1, 1, 1)),
            ])
            # r1B (12): h2/h3 q1
            weave(section((2, 3), 1), [
                (2, u_v(8)), (4, u_qk(0, 2, 1)),
                (6, u_v(9)), (8, u_qk(1, 2, 0)), (10, u_qk(1, 2, 1)),
            ])
            # r2A (16): h0/h1 q2
            weave(section((0, 1), 2), [
                (0, u_qk(0, 3, 0)), (2, u_v(10)), (4, u_v(11)),
                (8, u_qk(0, 3, 1)),
            ])
            # r2B (16): h2/h3 q2
            weave(section((2, 3), 2), [
                (4, u_qk(1, 3, 0)),
            ])
            # r3A (20): h0/h1 q3; v12-15 due before h0 kb12-15 AVs (~c16+)
            weave(section((0, 1), 3), [
                (0, u_v(12)), (2, u_v(13)), (4, u_v(14)), (6, u_v(15)),
            ])
            # r3B (20): h2/h3 q3; p1 k3 due before h2 kb12 (chunk 12)
            weave(section((2, 3), 3), [
                (6, u_qk(1, 3, 1)),
            ])
            attn_drain()
            pp.release()
            op.release()
            stp.release()


def _get_program():
    if "nc" not in _CACHE:
        _CACHE["nc"] = _build_program()
    return _CACHE["nc"]


def make_in_maps(x, Wqk, bqk, Wv, bv):
    ii, jj = np.meshgrid(np.arange(128), np.arange(128), indexing="ij")
    amask = np.where(ii <= jj, 1.0, 0.0).astype(BF16)
    in_maps = []
    for c in range(NCORES):
        b, g = divmod(c, 4)
        cols = slice(256 * g, 256 * (g + 1))
        xt = np.ascontiguousarray(x[b].T).astype(BF16).reshape(8, 128, S)
        wq = np.ascontiguousarray(Wqk[:, cols]).astype(BF16).reshape(8, 128, 256)
        wk = np.ascontiguousarray(Wqk[:, D:][:, cols]).astype(BF16).reshape(8, 128, 256)
        wv = np.ascontiguousarray(Wv[:, cols]).astype(BF16).reshape(8, 128, 256)
        bq = np.ascontiguousarray(bqk[cols].reshape(2, 128).T).astype(np.float32)
        bk = np.ascontiguousarray(bqk[D:][cols].reshape(2, 128).T).astype(np.float32)
        in_maps.append({"xt": xt, "wq": wq, "wk": wk, "wv": wv,
                        "bq": bq, "bk": bk, "amask": amask,
                        f"salt_{_kernel_salt()}": np.zeros((1, 2), np.float32)})
    return in_maps


def assemble(per_core_out, bv):
    out = np.empty((B, S, H * DVH), np.float32)
    for c in range(NCORES):
        b, g = divmod(c, 4)
        o = per_core_out[c]  # [HPC, 65, S]
        for hh in range(HPC):
            hg = HPC * g + hh
            a = o[hh, :64, :] / o[hh, 64:65, :]
            out[b, :, DVH * hg:DVH * (hg + 1)] = a.T + bv[DVH * hg:DVH * (hg + 1)]
    return out


def kernel(x, Wqk, bqk, Wv, bv):
    from concourse.bass_utils import run_bass_kernel_spmd

    nc = _get_program()
    in_maps = make_in_maps(np.asarray(x, np.float32), np.asarray(Wqk, np.float32),
                           np.asarray(bqk, np.float32), np.asarray(Wv, np.float32),
                           np.asarray(bv, np.float32))
    trace = os.environ.get("MHA_TRACE", "0") == "1"
    try:
        res = run_bass_kernel_spmd(nc, in_maps, list(range(NCORES)), trace=trace)
    except Exception:
        if not trace:
            raise
        # trace path unavailable (e.g. no NTFF hook on this axon client)
        res = run_bass_kernel_spmd(nc, in_maps, list(range(NCORES)), trace=False)
    _CACHE["last_result"] = res
    return assemble([r["out"] for r in res.results], np.asarray(bv, np.float32))

